# revision 41
# baseline (speedup 1.0000x reference)
"""DistanceAttention Trainium2 kernel.

Computes, for x:[B,T,D]:
    v    = x @ W_in.T + b_in
    attn = exp((-|i-j| + padding_mask) / e)        # [B,T,T], no softmax
    out  = attn @ v

Key facts exploited:
  * attn factors as exp(-|i-j|/e) * exp(mask_j/e).  The distance kernel
    r^|i-j| (r = exp(-1/e) ~= 0.692) underflows fp32 (< 1e-21) for
    |i-j| >= 128, so attn is numerically block-tridiagonal with three
    CONSTANT 128x128 blocks shared by every row-block/batch/core: the
    t x t matmul collapses to 3 small matmuls per 128-row block.
  * exp(mask/e) is a per-row scale of v and commutes with the
    projection: it is folded into x on the host.  Phantom halo rows are
    zero-padded, which the same mechanism handles.
  * b_in enters the output as (attn @ exp(mask/e)) (x) b_in -- a rank-1
    term added exactly on the host (b_in is zero here; generality path).

Sharding: batch(4) x seq-half(2) -> 8 cores, each owning 2048 rows plus
a 128-row halo per side.  No cross-core communication.
"""

import numpy as np

B, T, D = 4, 4096, 256
NCORES = 8
THALF = T // 2  # rows owned per core
HALO = 128
LOC = THALF + 2 * HALO  # local rows incl. halo
NBLK = LOC // 128  # 18 local 128-row blocks
# xT DMA chunk sizes in 128-row blocks: tiny first chunk unblocks the
# PE early, big chunks stream behind the first projections
CHUNKS = (2, 4, 12)
NCH = len(CHUNKS)
CHOFF = tuple(sum(CHUNKS[:j]) for j in range(NCH))
E = float(np.e)

# "f32r" streams fp32 data through the PE in single-pass mode (4x the
# throughput of the 2-pass fp32 decomposition); "f32" is the safe path.
MM_DTYPE = "f32r"

_CACHE: dict = {}


def _decay_blocks() -> np.ndarray:
    """lhsT-layout decay blocks [128, 3*128]: L | 0 | R.

    matmul(out, lhsT, rhs) computes out[p,n] = sum_q lhsT[q,p] rhs[q,n].
    Out-block m needs  A_L @ v[m-1] + A_0 @ v[m] + A_R @ v[m+1]  with
      A_L[p,q] = r^(128+p-q),  A_0[p,q] = r^|p-q|,  A_R[p,q] = r^(128+q-p)
    so lhsT_L[q,p] = A_L[p,q] etc.  Entries are computed exactly like the
    reference: exp(-dist/e) in fp32.
    """
    i = np.arange(128, dtype=np.float64)
    dL = 128.0 + i[None, :] - i[:, None]  # lhsT_L[a,b] = r^(128+b-a)
    d0 = np.abs(i[:, None] - i[None, :])
    dR = 128.0 + i[:, None] - i[None, :]  # lhsT_R[a,b] = r^(128+a-b)
    dist = np.concatenate([dL, d0, dR], axis=1)
    tg = (-dist.astype(np.float32)) / np.float32(E)
    return np.exp(tg).astype(np.float32)


def _build():
    import concourse.bacc as bacc
    import concourse.mybir as mybir
    from concourse.bass import ts
    from concourse.tile import TileContext

    fp = mybir.dt.float32
    mmdt = mybir.dt.float32r if MM_DTYPE == "f32r" else mybir.dt.float32

    nc = bacc.Bacc(None, target_bir_lowering=False, debug=False)

    # xT[k]: d-half k of the (pre-scaled, transposed) x slice
    xT = nc.dram_tensor("xT", [2, 128, LOC], mmdt, kind="ExternalInput")
    wT = nc.dram_tensor("wT", [2, 128, D], mmdt, kind="ExternalInput")
    out = nc.dram_tensor("out", [THALF, D], fp, kind="ExternalOutput")
    md = nc.dram_tensor("mdecay", [128, 3 * 128], mmdt, kind="ExternalInput")

    with TileContext(nc) as tc:
        with (
            tc.tile_pool(name="const", bufs=1) as cpool,
            tc.tile_pool(name="vpool", bufs=1) as vpool,
            tc.tile_pool(name="opool", bufs=3) as opool,
            tc.tile_pool(name="ppsum", bufs=3, space="PSUM") as ppsum,
            tc.tile_pool(name="dpsum", bufs=4, space="PSUM") as dpsum,
        ):
            # PE warmup: dummy matmuls with no data deps run during the
            # DMA lead so the HAM clock gate is at 8/8 (2.4 GHz) by the
            # time the first real matmul issues (~3.4us busy to warm)
            scr_w = cpool.tile([128, 128], fp, tag="scr_w")
            nc.vector.memset(scr_w[:], 0.0)
            scr_x = cpool.tile([128, 2 * D], fp, tag="scr_x")
            nc.vector.memset(scr_x[:], 0.0)
            wpsum = ppsum.tile([128, 2 * D], fp, tag="warm", bufs=1)
            for _ in range(3):
                nc.tensor.matmul(wpsum[:], scr_w[:], scr_x[:],
                                 start=True, stop=True)

            # DMA order = dependency order of the first matmuls; all on
            # one HWDGE queue -- serial issue naturally prioritizes the
            # early critical transfers over the big later chunks (a
            # parallel-queue split was measured slower: every transfer
            # then contends for HBM bandwidth at once).
            wT_sb = [None, None]
            xc = [[None] * NCH for _ in range(2)]
            for k in range(2):
                wT_sb[k] = cpool.tile([128, D], mmdt,
                                      name=f"w{k}", tag=f"w{k}")
                nc.sync.dma_start(out=wT_sb[k][:], in_=wT[k])

            def xslice(k, j):
                return xT[k][:, CHOFF[j] * 128:(CHOFF[j] + CHUNKS[j]) * 128]

            for k in range(2):
                xc[k][0] = cpool.tile([128, CHUNKS[0] * 128], mmdt,
                                      name=f"x{k}0", tag=f"x{k}0")
                nc.sync.dma_start(out=xc[k][0][:], in_=xslice(k, 0))
            md_sb = cpool.tile([128, 3 * 128], mmdt, tag="md")
            nc.sync.dma_start(out=md_sb[:], in_=md[:])
            for j in range(1, NCH):
                for k in range(2):
                    xc[k][j] = cpool.tile([128, CHUNKS[j] * 128], mmdt,
                                          name=f"x{k}{j}", tag=f"x{k}{j}")
                    nc.sync.dma_start(out=xc[k][j][:], in_=xslice(k, j))

            # all 18 v blocks in one tile so any 512-wide window
            # [v_a | v_a+1] is a contiguous rhs
            v_sb = vpool.tile([128, NBLK * D], mmdt, tag="v")

            def xap(k, m):  # lhsT for t-block m, d-half k
                j = max(jj for jj in range(NCH) if CHOFF[jj] <= m)
                return xc[k][j][:, ts(m - CHOFF[j], 128)]

            def proj_pair(p):
                # project blocks (2p, 2p+1) into one [128, 512] PSUM pair
                a = 2 * p
                pp = ppsum.tile([128, 2 * D], fp, tag="pp")
                nc.tensor.matmul(pp[:, 0:D], xap(0, a), wT_sb[0][:],
                                 start=True, stop=False)
                nc.tensor.matmul(pp[:, D:2 * D], xap(0, a + 1), wT_sb[0][:],
                                 start=False, stop=False)
                nc.tensor.matmul(pp[:, 0:D], xap(1, a), wT_sb[1][:],
                                 start=False, stop=False)
                nc.tensor.matmul(pp[:, D:2 * D], xap(1, a + 1), wT_sb[1][:],
                                 start=False, stop=True)
                nc.vector.tensor_copy(v_sb[:, a * D:(a + 2) * D], pp[:])

            def decay_pair(a):
                # out blocks (a, a+1) as one [128, 512] PSUM pair:
                # each diagonal's weights apply to both halves at once
                dp = dpsum.tile([128, 2 * D], fp, tag="dp")
                nc.tensor.matmul(dp[:], md_sb[:, 0:128],
                                 v_sb[:, (a - 1) * D:(a + 1) * D],
                                 start=True, stop=False)
                nc.tensor.matmul(dp[:], md_sb[:, 128:256],
                                 v_sb[:, a * D:(a + 2) * D],
                                 start=False, stop=False)
                nc.tensor.matmul(dp[:], md_sb[:, 256:384],
                                 v_sb[:, (a + 1) * D:(a + 3) * D],
                                 start=False, stop=True)
                dst = out.rearrange("(n p) d -> p n d", p=128)[:, a - 1:a + 1, :]
                ob = opool.tile([128, 2 * D], fp, tag="ob")
                # out-copies go to the otherwise-idle scalar engine so
                # the DVE (whose casts gate the decay matmuls) never
                # queues a non-critical copy ahead of a critical cast
                nc.scalar.copy(ob[:], dp[:])
                nc.sync.dma_start(
                    out=dst, in_=ob[:].rearrange("p (n d) -> p n d", n=2))

            # interleave: decay pair a=2k+1 (v blocks a-1..a+2) becomes
            # ready right after proj pair k+1 -- emit it there so its
            # copy/DMA drain while later projections still run
            proj_pair(0)
            proj_pair(1)
            decay_pair(1)
            for p in range(2, NBLK // 2):
                proj_pair(p)
                decay_pair(2 * p - 1)

    nc.compile()
    return nc


def _shard_inputs(x, padding_mask, W_in, b_in):
    x = np.asarray(x, np.float32)
    padding_mask = np.asarray(padding_mask, np.float32)
    if np.any(padding_mask):
        x = x * np.exp(padding_mask / np.float32(E)).transpose(0, 2, 1)
    wT = np.ascontiguousarray(np.asarray(W_in, np.float32).T).reshape(2, 128, D)
    mdec = _decay_blocks()
    in_maps = []
    for c in range(NCORES):
        bidx, half = divmod(c, 2)
        start = half * THALF
        lo, hi = start - HALO, start + THALF + HALO
        glo, ghi = max(lo, 0), min(hi, T)
        xsl = np.zeros((LOC, D), np.float32)
        xsl[glo - lo:ghi - lo] = x[bidx, glo:ghi]
        xTc = np.ascontiguousarray(xsl.T).reshape(2, 128, LOC)
        in_maps.append({"xT": xTc, "wT": wT, "mdecay": mdec})
    return in_maps


def _bias_correction(out, padding_mask, b_in):
    """out += attn @ (1 (x) b_in) = (attn_dist @ exp(mask/e)) (x) b_in."""
    b_in = np.asarray(b_in, np.float32)
    if not np.any(b_in):
        return
    k = np.arange(-256, 257, dtype=np.float32)
    w = np.exp(-np.abs(k) / np.float32(E)).astype(np.float64)
    s_all = np.exp(np.asarray(padding_mask, np.float32)[:, 0, :]
                   / np.float32(E)).astype(np.float64)
    for bidx in range(B):
        a = np.convolve(s_all[bidx], w, mode="same").astype(np.float32)
        out[bidx] += np.outer(a, b_in)


def kernel(x, padding_mask, W_in, b_in):
    from concourse.bass_utils import run_bass_kernel_spmd

    if "nc" not in _CACHE:
        _CACHE["nc"] = _build()
    nc = _CACHE["nc"]

    in_maps = _shard_inputs(x, padding_mask, W_in, b_in)
    res = run_bass_kernel_spmd(nc, in_maps, list(range(NCORES)))
    out = np.empty((B, T, D), np.float32)
    for c in range(NCORES):
        bidx, half = divmod(c, 2)
        out[bidx, half * THALF:(half + 1) * THALF] = res.results[c]["out"]
    _bias_correction(out, padding_mask, b_in)
    return out


# revision 42
# speedup vs baseline: 1.0319x; 1.0319x over previous
"""DistanceAttention Trainium2 kernel.

Computes, for x:[B,T,D]:
    v    = x @ W_in.T + b_in
    attn = exp((-|i-j| + padding_mask) / e)        # [B,T,T], no softmax
    out  = attn @ v

Key facts exploited:
  * attn factors as exp(-|i-j|/e) * exp(mask_j/e).  The distance kernel
    r^|i-j| (r = exp(-1/e) ~= 0.692) underflows fp32 (< 1e-21) for
    |i-j| >= 128, so attn is numerically block-tridiagonal with three
    CONSTANT 128x128 blocks shared by every row-block/batch/core: the
    t x t matmul collapses to 3 small matmuls per 128-row block.
  * exp(mask/e) is a per-row scale of v and commutes with the
    projection: it is folded into x on the host.  Phantom halo rows are
    zero-padded, which the same mechanism handles.
  * b_in enters the output as (attn @ exp(mask/e)) (x) b_in -- a rank-1
    term added exactly on the host (b_in is zero here; generality path).

Sharding: batch(4) x seq-half(2) -> 8 cores, each owning 2048 rows plus
a 128-row halo per side.  No cross-core communication.
"""

import numpy as np

B, T, D = 4, 4096, 256
NCORES = 8
THALF = T // 2  # rows owned per core
HALO = 128
LOC = THALF + 2 * HALO  # local rows incl. halo
NBLK = LOC // 128  # 18 local 128-row blocks
# xT DMA chunk sizes in 128-row blocks: tiny first chunk unblocks the
# PE early, big chunks stream behind the first projections
CHUNKS = (2, 4, 6, 6)
NCH = len(CHUNKS)
CHOFF = tuple(sum(CHUNKS[:j]) for j in range(NCH))
E = float(np.e)

# "f32r" streams fp32 data through the PE in single-pass mode (4x the
# throughput of the 2-pass fp32 decomposition); "f32" is the safe path.
MM_DTYPE = "f32r"

_CACHE: dict = {}


def _decay_blocks() -> np.ndarray:
    """lhsT-layout decay blocks [128, 3*128]: L | 0 | R.

    matmul(out, lhsT, rhs) computes out[p,n] = sum_q lhsT[q,p] rhs[q,n].
    Out-block m needs  A_L @ v[m-1] + A_0 @ v[m] + A_R @ v[m+1]  with
      A_L[p,q] = r^(128+p-q),  A_0[p,q] = r^|p-q|,  A_R[p,q] = r^(128+q-p)
    so lhsT_L[q,p] = A_L[p,q] etc.  Entries are computed exactly like the
    reference: exp(-dist/e) in fp32.
    """
    i = np.arange(128, dtype=np.float64)
    dL = 128.0 + i[None, :] - i[:, None]  # lhsT_L[a,b] = r^(128+b-a)
    d0 = np.abs(i[:, None] - i[None, :])
    dR = 128.0 + i[:, None] - i[None, :]  # lhsT_R[a,b] = r^(128+a-b)
    dist = np.concatenate([dL, d0, dR], axis=1)
    tg = (-dist.astype(np.float32)) / np.float32(E)
    return np.exp(tg).astype(np.float32)


def _build():
    import concourse.bacc as bacc
    import concourse.mybir as mybir
    from concourse.bass import ts
    from concourse.tile import TileContext

    fp = mybir.dt.float32
    mmdt = mybir.dt.float32r if MM_DTYPE == "f32r" else mybir.dt.float32

    nc = bacc.Bacc(None, target_bir_lowering=False, debug=False)

    # xT[k]: d-half k of the (pre-scaled, transposed) x slice
    xT = nc.dram_tensor("xT", [2, 128, LOC], mmdt, kind="ExternalInput")
    wT = nc.dram_tensor("wT", [2, 128, D], mmdt, kind="ExternalInput")
    out = nc.dram_tensor("out", [THALF, D], fp, kind="ExternalOutput")
    md = nc.dram_tensor("mdecay", [128, 3 * 128], mmdt, kind="ExternalInput")

    with TileContext(nc) as tc:
        with (
            tc.tile_pool(name="const", bufs=1) as cpool,
            tc.tile_pool(name="vpool", bufs=1) as vpool,
            tc.tile_pool(name="opool", bufs=3) as opool,
            tc.tile_pool(name="ppsum", bufs=3, space="PSUM") as ppsum,
            tc.tile_pool(name="dpsum", bufs=4, space="PSUM") as dpsum,
        ):
            # PE warmup: dummy matmuls with no data deps run during the
            # DMA lead so the HAM clock gate is at 8/8 (2.4 GHz) by the
            # time the first real matmul issues (~3.4us busy to warm)
            scr_w = cpool.tile([128, 128], fp, tag="scr_w")
            nc.vector.memset(scr_w[:], 0.0)
            scr_x = cpool.tile([128, 2 * D], fp, tag="scr_x")
            nc.vector.memset(scr_x[:], 0.0)
            wpsum = ppsum.tile([128, 2 * D], fp, tag="warm", bufs=1)
            for _ in range(3):
                nc.tensor.matmul(wpsum[:], scr_w[:], scr_x[:],
                                 start=True, stop=True)

            # DMA order = dependency order of the first matmuls; all on
            # one HWDGE queue -- serial issue naturally prioritizes the
            # early critical transfers over the big later chunks (a
            # parallel-queue split was measured slower: every transfer
            # then contends for HBM bandwidth at once).
            wT_sb = [None, None]
            xc = [[None] * NCH for _ in range(2)]
            for k in range(2):
                wT_sb[k] = cpool.tile([128, D], mmdt,
                                      name=f"w{k}", tag=f"w{k}")
                nc.sync.dma_start(out=wT_sb[k][:], in_=wT[k])

            def xslice(k, j):
                return xT[k][:, CHOFF[j] * 128:(CHOFF[j] + CHUNKS[j]) * 128]

            for k in range(2):
                xc[k][0] = cpool.tile([128, CHUNKS[0] * 128], mmdt,
                                      name=f"x{k}0", tag=f"x{k}0")
                nc.sync.dma_start(out=xc[k][0][:], in_=xslice(k, 0))
            md_sb = cpool.tile([128, 3 * 128], mmdt, tag="md")
            nc.sync.dma_start(out=md_sb[:], in_=md[:])
            for j in range(1, NCH):
                for k in range(2):
                    xc[k][j] = cpool.tile([128, CHUNKS[j] * 128], mmdt,
                                          name=f"x{k}{j}", tag=f"x{k}{j}")
                    nc.sync.dma_start(out=xc[k][j][:], in_=xslice(k, j))

            # all 18 v blocks in one tile so any 512-wide window
            # [v_a | v_a+1] is a contiguous rhs
            v_sb = vpool.tile([128, NBLK * D], mmdt, tag="v")

            def xap(k, m):  # lhsT for t-block m, d-half k
                j = max(jj for jj in range(NCH) if CHOFF[jj] <= m)
                return xc[k][j][:, ts(m - CHOFF[j], 128)]

            def proj_pair(p):
                # project blocks (2p, 2p+1) into one [128, 512] PSUM pair
                a = 2 * p
                pp = ppsum.tile([128, 2 * D], fp, tag="pp")
                nc.tensor.matmul(pp[:, 0:D], xap(0, a), wT_sb[0][:],
                                 start=True, stop=False)
                nc.tensor.matmul(pp[:, D:2 * D], xap(0, a + 1), wT_sb[0][:],
                                 start=False, stop=False)
                nc.tensor.matmul(pp[:, 0:D], xap(1, a), wT_sb[1][:],
                                 start=False, stop=False)
                nc.tensor.matmul(pp[:, D:2 * D], xap(1, a + 1), wT_sb[1][:],
                                 start=False, stop=True)
                nc.vector.tensor_copy(v_sb[:, a * D:(a + 2) * D], pp[:])

            def decay_pair(a):
                # out blocks (a, a+1) as one [128, 512] PSUM pair:
                # each diagonal's weights apply to both halves at once
                dp = dpsum.tile([128, 2 * D], fp, tag="dp")
                nc.tensor.matmul(dp[:], md_sb[:, 0:128],
                                 v_sb[:, (a - 1) * D:(a + 1) * D],
                                 start=True, stop=False)
                nc.tensor.matmul(dp[:], md_sb[:, 128:256],
                                 v_sb[:, a * D:(a + 2) * D],
                                 start=False, stop=False)
                nc.tensor.matmul(dp[:], md_sb[:, 256:384],
                                 v_sb[:, (a + 1) * D:(a + 3) * D],
                                 start=False, stop=True)
                dst = out.rearrange("(n p) d -> p n d", p=128)[:, a - 1:a + 1, :]
                ob = opool.tile([128, 2 * D], fp, tag="ob")
                # out-copies go to the otherwise-idle scalar engine so
                # the DVE (whose casts gate the decay matmuls) never
                # queues a non-critical copy ahead of a critical cast
                nc.scalar.copy(ob[:], dp[:])
                nc.sync.dma_start(
                    out=dst, in_=ob[:].rearrange("p (n d) -> p n d", n=2))

            # interleave: decay pair a=2k+1 (v blocks a-1..a+2) becomes
            # ready right after proj pair k+1 -- emit it there so its
            # copy/DMA drain while later projections still run
            proj_pair(0)
            proj_pair(1)
            decay_pair(1)
            for p in range(2, NBLK // 2):
                proj_pair(p)
                decay_pair(2 * p - 1)

    nc.compile()
    return nc


def _shard_inputs(x, padding_mask, W_in, b_in):
    x = np.asarray(x, np.float32)
    padding_mask = np.asarray(padding_mask, np.float32)
    if np.any(padding_mask):
        x = x * np.exp(padding_mask / np.float32(E)).transpose(0, 2, 1)
    wT = np.ascontiguousarray(np.asarray(W_in, np.float32).T).reshape(2, 128, D)
    mdec = _decay_blocks()
    in_maps = []
    for c in range(NCORES):
        bidx, half = divmod(c, 2)
        start = half * THALF
        lo, hi = start - HALO, start + THALF + HALO
        glo, ghi = max(lo, 0), min(hi, T)
        xsl = np.zeros((LOC, D), np.float32)
        xsl[glo - lo:ghi - lo] = x[bidx, glo:ghi]
        xTc = np.ascontiguousarray(xsl.T).reshape(2, 128, LOC)
        in_maps.append({"xT": xTc, "wT": wT, "mdecay": mdec})
    return in_maps


def _bias_correction(out, padding_mask, b_in):
    """out += attn @ (1 (x) b_in) = (attn_dist @ exp(mask/e)) (x) b_in."""
    b_in = np.asarray(b_in, np.float32)
    if not np.any(b_in):
        return
    k = np.arange(-256, 257, dtype=np.float32)
    w = np.exp(-np.abs(k) / np.float32(E)).astype(np.float64)
    s_all = np.exp(np.asarray(padding_mask, np.float32)[:, 0, :]
                   / np.float32(E)).astype(np.float64)
    for bidx in range(B):
        a = np.convolve(s_all[bidx], w, mode="same").astype(np.float32)
        out[bidx] += np.outer(a, b_in)


def kernel(x, padding_mask, W_in, b_in):
    from concourse.bass_utils import run_bass_kernel_spmd

    if "nc" not in _CACHE:
        _CACHE["nc"] = _build()
    nc = _CACHE["nc"]

    in_maps = _shard_inputs(x, padding_mask, W_in, b_in)
    res = run_bass_kernel_spmd(nc, in_maps, list(range(NCORES)))
    out = np.empty((B, T, D), np.float32)
    for c in range(NCORES):
        bidx, half = divmod(c, 2)
        out[bidx, half * THALF:(half + 1) * THALF] = res.results[c]["out"]
    _bias_correction(out, padding_mask, b_in)
    return out


# revision 46
# speedup vs baseline: 1.1385x; 1.1034x over previous
"""DistanceAttention Trainium2 kernel.

Computes, for x:[B,T,D]:
    v    = x @ W_in.T + b_in
    attn = exp((-|i-j| + padding_mask) / e)        # [B,T,T], no softmax
    out  = attn @ v

Key facts exploited:
  * attn factors as exp(-|i-j|/e) * exp(mask_j/e).  The distance kernel
    r^|i-j| (r = exp(-1/e) ~= 0.692) underflows fp32 (< 1e-21) for
    |i-j| >= 128, so attn is numerically block-tridiagonal with three
    CONSTANT 128x128 blocks shared by every row-block/batch/core: the
    t x t matmul collapses to 3 small matmuls per 128-row block.
  * exp(mask/e) is a per-row scale of v and commutes with the
    projection: it is folded into x on the host.  Phantom halo rows are
    zero-padded, which the same mechanism handles.
  * b_in enters the output as (attn @ exp(mask/e)) (x) b_in -- a rank-1
    term added exactly on the host (b_in is zero here; generality path).

Sharding: batch(4) x seq-half(2) -> 8 cores, each owning 2048 rows plus
a 128-row halo per side.  No cross-core communication.
"""

import numpy as np

B, T, D = 4, 4096, 256
NCORES = 8
THALF = T // 2  # rows owned per core
HALO = 128
LOC = THALF + 2 * HALO  # local rows incl. halo
NBLK = LOC // 128  # 18 local 128-row blocks
# xT DMA chunk sizes in 128-row blocks: tiny first chunk unblocks the
# PE early, big chunks stream behind the first projections
CHUNKS = (2, 4, 6, 6)
NCH = len(CHUNKS)
CHOFF = tuple(sum(CHUNKS[:j]) for j in range(NCH))
E = float(np.e)

# "f32r" streams fp32 data through the PE in single-pass mode (4x the
# throughput of the 2-pass fp32 decomposition); "f32" is the safe path.
MM_DTYPE = "f32r"

_CACHE: dict = {}


def _decay_blocks() -> np.ndarray:
    """lhsT-layout decay blocks [128, 3*128]: L | 0 | R.

    matmul(out, lhsT, rhs) computes out[p,n] = sum_q lhsT[q,p] rhs[q,n].
    Out-block m needs  A_L @ v[m-1] + A_0 @ v[m] + A_R @ v[m+1]  with
      A_L[p,q] = r^(128+p-q),  A_0[p,q] = r^|p-q|,  A_R[p,q] = r^(128+q-p)
    so lhsT_L[q,p] = A_L[p,q] etc.  Entries are computed exactly like the
    reference: exp(-dist/e) in fp32.
    """
    i = np.arange(128, dtype=np.float64)
    dL = 128.0 + i[None, :] - i[:, None]  # lhsT_L[a,b] = r^(128+b-a)
    d0 = np.abs(i[:, None] - i[None, :])
    dR = 128.0 + i[:, None] - i[None, :]  # lhsT_R[a,b] = r^(128+a-b)
    dist = np.concatenate([dL, d0, dR], axis=1)
    tg = (-dist.astype(np.float32)) / np.float32(E)
    return np.exp(tg).astype(np.float32)


def _build():
    import concourse.bacc as bacc
    import concourse.mybir as mybir
    from concourse.bass import ts
    from concourse.tile import TileContext

    fp = mybir.dt.float32
    mmdt = mybir.dt.float32r if MM_DTYPE == "f32r" else mybir.dt.float32

    nc = bacc.Bacc(None, target_bir_lowering=False, debug=False)

    # xT[k]: d-half k of the (pre-scaled, transposed) x slice
    xT = nc.dram_tensor("xT", [2, 128, LOC], mmdt, kind="ExternalInput")
    wT = nc.dram_tensor("wT", [2, 128, D], mmdt, kind="ExternalInput")
    out = nc.dram_tensor("out", [THALF, D], fp, kind="ExternalOutput")
    md = nc.dram_tensor("mdecay", [128, 3 * 128], mmdt, kind="ExternalInput")

    with TileContext(nc) as tc:
        with (
            tc.tile_pool(name="const", bufs=1) as cpool,
            tc.tile_pool(name="vpool", bufs=1) as vpool,
            tc.tile_pool(name="opool", bufs=3) as opool,
            tc.tile_pool(name="ppsum", bufs=3, space="PSUM") as ppsum,
            tc.tile_pool(name="dpsum", bufs=4, space="PSUM") as dpsum,
        ):
            # PE warmup: dummy matmuls with no data deps run during the
            # DMA lead so the HAM clock gate is at 8/8 (2.4 GHz) by the
            # time the first real matmul issues (~3.4us busy to warm)
            scr_w = cpool.tile([128, 128], fp, tag="scr_w")
            nc.vector.memset(scr_w[:], 0.0)
            scr_x = cpool.tile([128, 2 * D], fp, tag="scr_x")
            nc.vector.memset(scr_x[:], 0.0)
            wpsum = ppsum.tile([128, 2 * D], fp, tag="warm", bufs=1)
            for _ in range(3):
                nc.tensor.matmul(wpsum[:], scr_w[:], scr_x[:],
                                 start=True, stop=True)

            # DMA order = dependency order of the first matmuls; all on
            # one HWDGE queue -- serial issue naturally prioritizes the
            # early critical transfers over the big later chunks (a
            # parallel-queue split was measured slower: every transfer
            # then contends for HBM bandwidth at once).  Each chunk's
            # two d-halves ride ONE 3D-AP DMA into a packed tile to
            # halve the ~650ns-per-DMA issue serialization.
            wb = cpool.tile([128, 2 * D], mmdt, tag="wb")
            nc.sync.dma_start(out=wb[:].rearrange("p (k c) -> p k c", k=2),
                              in_=wT[:].rearrange("k p c -> p k c"))
            wT_sb = [wb[:, 0:D], wb[:, D:2 * D]]

            xb = [None] * NCH

            def xchunk_dma(j):
                t = cpool.tile([128, 2 * CHUNKS[j] * 128], mmdt,
                               name=f"xb{j}", tag=f"xb{j}")
                sl = xT[:, :, CHOFF[j] * 128:(CHOFF[j] + CHUNKS[j]) * 128]
                nc.sync.dma_start(out=t[:].rearrange("p (k c) -> p k c", k=2),
                                  in_=sl.rearrange("k p c -> p k c"))
                xb[j] = t

            xchunk_dma(0)
            md_sb = cpool.tile([128, 3 * 128], mmdt, tag="md")
            nc.sync.dma_start(out=md_sb[:], in_=md[:])
            for j in range(1, NCH):
                xchunk_dma(j)

            # all 18 v blocks in one tile so any 512-wide window
            # [v_a | v_a+1] is a contiguous rhs
            v_sb = vpool.tile([128, NBLK * D], mmdt, tag="v")

            def xap(k, m):  # lhsT for t-block m, d-half k
                j = max(jj for jj in range(NCH) if CHOFF[jj] <= m)
                return xb[j][:, ts(k * CHUNKS[j] + m - CHOFF[j], 128)]

            def proj_pair(p):
                # project blocks (2p, 2p+1) into one [128, 512] PSUM pair
                a = 2 * p
                pp = ppsum.tile([128, 2 * D], fp, tag="pp")
                nc.tensor.matmul(pp[:, 0:D], xap(0, a), wT_sb[0][:],
                                 start=True, stop=False)
                nc.tensor.matmul(pp[:, D:2 * D], xap(0, a + 1), wT_sb[0][:],
                                 start=False, stop=False)
                nc.tensor.matmul(pp[:, 0:D], xap(1, a), wT_sb[1][:],
                                 start=False, stop=False)
                nc.tensor.matmul(pp[:, D:2 * D], xap(1, a + 1), wT_sb[1][:],
                                 start=False, stop=True)
                nc.vector.tensor_copy(v_sb[:, a * D:(a + 2) * D], pp[:])

            def decay_pair(a):
                # out blocks (a, a+1) as one [128, 512] PSUM pair:
                # each diagonal's weights apply to both halves at once
                dp = dpsum.tile([128, 2 * D], fp, tag="dp")
                nc.tensor.matmul(dp[:], md_sb[:, 0:128],
                                 v_sb[:, (a - 1) * D:(a + 1) * D],
                                 start=True, stop=False)
                nc.tensor.matmul(dp[:], md_sb[:, 128:256],
                                 v_sb[:, a * D:(a + 2) * D],
                                 start=False, stop=False)
                nc.tensor.matmul(dp[:], md_sb[:, 256:384],
                                 v_sb[:, (a + 1) * D:(a + 3) * D],
                                 start=False, stop=True)
                dst = out.rearrange("(n p) d -> p n d", p=128)[:, a - 1:a + 1, :]
                ob = opool.tile([128, 2 * D], fp, tag="ob")
                # out-copies go to the otherwise-idle scalar engine so
                # the DVE (whose casts gate the decay matmuls) never
                # queues a non-critical copy ahead of a critical cast
                nc.scalar.copy(ob[:], dp[:])
                nc.sync.dma_start(
                    out=dst, in_=ob[:].rearrange("p (n d) -> p n d", n=2))

            # interleave: decay pair a=2k+1 (v blocks a-1..a+2) becomes
            # ready right after proj pair k+1 -- emit it there so its
            # copy/DMA drain while later projections still run
            proj_pair(0)
            proj_pair(1)
            decay_pair(1)
            for p in range(2, NBLK // 2):
                proj_pair(p)
                decay_pair(2 * p - 1)

    nc.compile()
    return nc


def _shard_inputs(x, padding_mask, W_in, b_in):
    x = np.asarray(x, np.float32)
    padding_mask = np.asarray(padding_mask, np.float32)
    if np.any(padding_mask):
        x = x * np.exp(padding_mask / np.float32(E)).transpose(0, 2, 1)
    wT = np.ascontiguousarray(np.asarray(W_in, np.float32).T).reshape(2, 128, D)
    mdec = _decay_blocks()
    in_maps = []
    for c in range(NCORES):
        bidx, half = divmod(c, 2)
        start = half * THALF
        lo, hi = start - HALO, start + THALF + HALO
        glo, ghi = max(lo, 0), min(hi, T)
        xsl = np.zeros((LOC, D), np.float32)
        xsl[glo - lo:ghi - lo] = x[bidx, glo:ghi]
        xTc = np.ascontiguousarray(xsl.T).reshape(2, 128, LOC)
        in_maps.append({"xT": xTc, "wT": wT, "mdecay": mdec})
    return in_maps


def _bias_correction(out, padding_mask, b_in):
    """out += attn @ (1 (x) b_in) = (attn_dist @ exp(mask/e)) (x) b_in."""
    b_in = np.asarray(b_in, np.float32)
    if not np.any(b_in):
        return
    k = np.arange(-256, 257, dtype=np.float32)
    w = np.exp(-np.abs(k) / np.float32(E)).astype(np.float64)
    s_all = np.exp(np.asarray(padding_mask, np.float32)[:, 0, :]
                   / np.float32(E)).astype(np.float64)
    for bidx in range(B):
        a = np.convolve(s_all[bidx], w, mode="same").astype(np.float32)
        out[bidx] += np.outer(a, b_in)


def kernel(x, padding_mask, W_in, b_in):
    from concourse.bass_utils import run_bass_kernel_spmd

    if "nc" not in _CACHE:
        _CACHE["nc"] = _build()
    nc = _CACHE["nc"]

    in_maps = _shard_inputs(x, padding_mask, W_in, b_in)
    res = run_bass_kernel_spmd(nc, in_maps, list(range(NCORES)))
    out = np.empty((B, T, D), np.float32)
    for c in range(NCORES):
        bidx, half = divmod(c, 2)
        out[bidx, half * THALF:(half + 1) * THALF] = res.results[c]["out"]
    _bias_correction(out, padding_mask, b_in)
    return out
